# revision 8
# baseline (speedup 1.0000x reference)
"""Additive (Bahdanau) attention on 8 TRN2 NeuronCores.

Reference:
    e_proj = einsum('tne,ae->tna', enc_out, W_e)          # [T,N,A]
    d_proj = einsum('nd,ad->na', dec_h, W_d)[None]        # [1,N,A]
    scores = einsum('tna,a->tn', tanh(e_proj + d_proj), v[0])
    alpha  = softmax(scores, axis=0)                      # over T
    ctx    = einsum('tn,tne->ne', alpha, enc_out)
    return ctx, alpha

Sharding: batch N=64 split 8 ways (n=8 per core), weights replicated, softmax
over T is core-local.  Device work per core (bf16 compute, f32 accumulate):
  phase A: e_projT chunks = W_eT.T @ encT  (PSUM), tanh(+d_proj bias) on ACT,
           scores via matmul against v packed in one-hot M-columns so group g
           lands on PSUM partition g -> scores live as [4, 512] per n
  softmax: exp straight off PSUM (scores bounded ~+-4, no max shift needed),
           free-dim partial sums, cross-partition total via a tiny ones-matmul,
           normalize on DVE
  phase B: ctx = alpha-weighted sum over T as K=128 matmuls on enc (T-major);
           the [4,512] alpha reshapes to the [128,16] lhsT via a DRAM bounce
           (Tile tracks the RAW dep through DRAM)
Host side only reshapes/transposes/casts inputs and glues shard outputs.
"""

import sys

for _p in ("/opt/trn_rl_repo", "/opt/pypackages"):
    if _p not in sys.path:
        sys.path.append(_p)

import numpy as np
import ml_dtypes
from contextlib import ExitStack

T, N, E, D, A = 2048, 64, 1024, 1024, 512
NC = 8          # cores
NL = N // NC    # batch per core = 8
P = 128         # partitions
EC = E // P     # 8 E-chunks
AC = A // P     # 4 A-chunks
DC = D // P     # 8 D-chunks
TG = 512        # t-group size for phase A
NG = T // TG    # 4 groups
TCH = T // P    # 16 t-chunks for phase B (K=128 each)

BF16 = ml_dtypes.bfloat16

_compiled = None


def _build():
    import concourse.bass as bass
    import concourse.tile as tile
    from concourse import bacc, mybir

    bf = mybir.dt.bfloat16
    f32 = mybir.dt.float32
    Act = mybir.ActivationFunctionType
    ts = bass.ts

    nc = bacc.Bacc()

    encT = nc.declare_dram_parameter("encT", [E, NL, T], bf, isOutput=False)
    encN = nc.declare_dram_parameter("encN", [NL, T, E], bf, isOutput=False)
    wet = nc.declare_dram_parameter("wet", [P, EC * A], bf, isOutput=False)
    wdt = nc.declare_dram_parameter("wdt", [P, DC * A], bf, isOutput=False)
    dech = nc.declare_dram_parameter("dech", [P, DC * NL], bf, isOutput=False)
    # v packed per (g, ac): [128, 4] slice = v[ac-chunk] in column g, else 0,
    # so the scores matmul for t-group g writes PSUM partition g.
    vg = nc.declare_dram_parameter("vg", [P, NG * AC * NG], bf, isOutput=False)
    ctx_out = nc.declare_dram_parameter("ctx", [NL, E], f32, isOutput=True)
    alphaT_out = nc.declare_dram_parameter("alphaT", [NL, T], f32, isOutput=True)

    abd = nc.dram_tensor("abd", [NL, T], bf)  # alpha bounce for reshape

    with tile.TileContext(nc) as tc, ExitStack() as ctx:
        wpool = ctx.enter_context(tc.tile_pool(name="w", bufs=1))
        etp = ctx.enter_context(tc.tile_pool(name="etp", bufs=5))
        enp = ctx.enter_context(tc.tile_pool(name="enp", bufs=5))
        thp = ctx.enter_context(tc.tile_pool(name="thp", bufs=6))
        scp = ctx.enter_context(tc.tile_pool(name="scp", bufs=2))
        alp = ctx.enter_context(tc.tile_pool(name="alp", bufs=2))
        ppe = ctx.enter_context(
            tc.tile_pool(name="ppe", bufs=4, space=bass.MemorySpace.PSUM)
        )
        pps = ctx.enter_context(
            tc.tile_pool(name="pps", bufs=2, space=bass.MemorySpace.PSUM)
        )
        ppc = ctx.enter_context(
            tc.tile_pool(name="ppc", bufs=2, space=bass.MemorySpace.PSUM)
        )

        # weights split across the two HWDGE rings so the startup chain
        # parallelizes: ACT ring feeds the first e_proj matmuls, SP ring
        # feeds d_proj.
        wet_sb = wpool.tile([P, EC * A], bf)
        nc.scalar.dma_start(wet_sb[:], wet[:])
        vg_sb = wpool.tile([P, NG * AC * NG], bf)
        nc.scalar.dma_start(vg_sb[:], vg[:])
        wdt_sb = wpool.tile([P, DC * A], bf)
        nc.sync.dma_start(wdt_sb[:], wdt[:])
        dech_sb = wpool.tile([P, DC * NL], bf)
        nc.sync.dma_start(dech_sb[:], dech[:])

        ones4 = wpool.tile([4, 4], f32)
        nc.vector.memset(ones4[:], 1.0)

        # --- d_projT[a, n] = (dec_h @ W_d.T).T, chunk ac at cols ac*NL ---
        dproj_sb = wpool.tile([P, AC * NL], f32)
        for ac in range(AC):
            dps = ppc.tile([P, NL], f32, tag="ppc")
            for dc in range(DC):
                nc.tensor.matmul(
                    dps[:],
                    wdt_sb[:, dc * A + ac * P : dc * A + (ac + 1) * P],
                    dech_sb[:, dc * NL : (dc + 1) * NL],
                    start=(dc == 0),
                    stop=(dc == DC - 1),
                )
            nc.scalar.activation(
                dproj_sb[:, ac * NL : (ac + 1) * NL], dps[:], Act.Copy
            )

        def phase_a(n):
            """scores for all T of batch-row n -> [4, TG] PSUM tile."""
            sps4 = pps.tile([NG, TG], f32, tag="pps")
            for g in range(NG):
                et = etp.tile([P, EC * TG], bf, tag="etp")
                src = encT[:, n, ts(g, TG)].rearrange("(a p) t -> p a t", p=P)
                nc.scalar.dma_start(
                    et[:].rearrange("p (a t) -> p a t", t=TG), src
                )
                for ac in range(AC):
                    eps = ppe.tile([P, TG], f32, tag="ppe")
                    for ec in range(EC):
                        nc.tensor.matmul(
                            eps[:],
                            wet_sb[:, ec * A + ac * P : ec * A + (ac + 1) * P],
                            et[:, ts(ec, TG)],
                            start=(ec == 0),
                            stop=(ec == EC - 1),
                        )
                    th = thp.tile([P, TG], bf, tag="thp")
                    nc.scalar.activation(
                        th[:],
                        eps[:],
                        Act.Tanh,
                        bias=dproj_sb[:, ac * NL + n : ac * NL + n + 1],
                    )
                    nc.tensor.matmul(
                        sps4[:],
                        vg_sb[:, ts(g * AC + ac, NG)],
                        th[:],
                        start=(g == 0 and ac == 0),
                        stop=(g == NG - 1 and ac == AC - 1),
                    )
            return sps4

        def softmax(n, sps4):
            """alpha out to DRAM + [P, TCH] bf16 lhsT tile for phase B."""
            ex4 = scp.tile([NG, TG], f32, tag="ex")
            nc.scalar.activation(ex4[:], sps4[:], Act.Exp)
            l4 = alp.tile([NG, 1], f32, tag="l4")
            nc.vector.reduce_sum(l4[:], ex4[:], axis=mybir.AxisListType.X)
            ltp = ppc.tile([NG, 1], f32, tag="ppc")
            nc.tensor.matmul(ltp[:], ones4[:], l4[:], start=True, stop=True)
            lts = alp.tile([NG, 1], f32, tag="lts")
            nc.scalar.activation(lts[:], ltp[:], Act.Copy)
            r4 = alp.tile([NG, 1], f32, tag="r4")
            nc.vector.reciprocal(r4[:], lts[:])
            al4 = scp.tile([NG, TG], f32, tag="al")
            nc.vector.tensor_scalar_mul(al4[:], ex4[:], r4[:])
            nc.gpsimd.dma_start(
                alphaT_out[n].rearrange("(g j) -> g j", g=NG), al4[:]
            )
            ab4 = scp.tile([NG, TG], bf, tag="ab")
            nc.vector.tensor_scalar_mul(ab4[:], ex4[:], r4[:])
            # reshape [4, TG] -> [P, TCH] via DRAM bounce; Tile tracks the
            # RAW dep through DRAM (read waits on the write's DMA sem).
            lt = alp.tile([P, TCH], bf, tag="lt")
            nc.gpsimd.dma_start(
                abd[n].rearrange("(g j) -> g j", g=NG), ab4[:]
            )
            nc.gpsimd.dma_start(lt[:], abd[n].rearrange("(p c) -> p c", p=P))
            return lt

        def phase_b(n, lt):
            ens = []
            for g in range(NG):
                en = enp.tile([P, 4 * E], bf, tag="enp")
                src = encN[n].rearrange("(p c) e -> p c e", c=TCH)[
                    :, 4 * g : 4 * (g + 1), :
                ]
                nc.sync.dma_start(
                    en[:].rearrange("p (c e) -> p c e", e=E), src
                )
                ens.append(en)
            ctx_sb = alp.tile([1, E], f32, tag="cs")
            for h in range(2):
                cp = ppc.tile([1, 512], f32, tag="ppc")
                for c in range(TCH):
                    nc.tensor.matmul(
                        cp[:],
                        lt[:, c : c + 1],
                        ens[c // 4][
                            :, (c % 4) * E + h * 512 : (c % 4) * E + h * 512 + 512
                        ],
                        start=(c == 0),
                        stop=(c == TCH - 1),
                    )
                nc.scalar.activation(ctx_sb[0:1, ts(h, 512)], cp[:], Act.Copy)
            nc.gpsimd.dma_start(ctx_out[n : n + 1, :], ctx_sb[:])

        # software-pipelined by one n: PE order A(0) A(1) B(0) A(2) B(1) ...
        sps4 = phase_a(0)
        for n in range(NL):
            lt = softmax(n, sps4)
            if n + 1 < NL:
                sps4 = phase_a(n + 1)
            phase_b(n, lt)

    nc.compile()
    return nc


def _prep_core(enc_sl, dec_sl, W_e, W_d, v):
    """Build the per-core in_map (layout transforms + bf16 cast only)."""
    encT = np.ascontiguousarray(enc_sl.transpose(2, 1, 0)).astype(BF16)
    encN = np.ascontiguousarray(enc_sl.transpose(1, 0, 2)).astype(BF16)
    wet = np.ascontiguousarray(
        W_e.T.reshape(EC, P, A).transpose(1, 0, 2).reshape(P, EC * A)
    ).astype(BF16)
    wdt = np.ascontiguousarray(
        W_d.T.reshape(DC, P, A).transpose(1, 0, 2).reshape(P, DC * A)
    ).astype(BF16)
    dech = np.ascontiguousarray(
        dec_sl.T.reshape(DC, P, NL).transpose(1, 0, 2).reshape(P, DC * NL)
    ).astype(BF16)
    vcols = v[0].reshape(AC, P).T  # [128, ac]
    vgm = np.zeros((P, NG * AC * NG), dtype=np.float32)
    for g in range(NG):
        for ac in range(AC):
            vgm[:, (g * AC + ac) * NG + g] = vcols[:, ac]
    return {
        "encT": encT,
        "encN": encN,
        "wet": wet,
        "wdt": wdt,
        "dech": dech,
        "vg": vgm.astype(BF16),
    }


def kernel(enc_out, dec_h, W_e, W_d, v, _trace=False):
    global _compiled
    from concourse.bass_utils import run_bass_kernel_spmd

    if _compiled is None:
        _compiled = _build()
    nc = _compiled

    enc_out = np.asarray(enc_out, dtype=np.float32)
    dec_h = np.asarray(dec_h, dtype=np.float32)
    W_e = np.asarray(W_e, dtype=np.float32)
    W_d = np.asarray(W_d, dtype=np.float32)
    v = np.asarray(v, dtype=np.float32)

    in_maps = [
        _prep_core(
            enc_out[:, c * NL : (c + 1) * NL, :],
            dec_h[c * NL : (c + 1) * NL],
            W_e,
            W_d,
            v,
        )
        for c in range(NC)
    ]

    res = run_bass_kernel_spmd(nc, in_maps, list(range(NC)), trace=_trace)
    ctx = np.concatenate([np.asarray(r["ctx"]) for r in res.results], axis=0)
    alpha = np.concatenate(
        [np.asarray(r["alphaT"]).T for r in res.results], axis=1
    )
    if _trace:
        return (ctx, alpha), res
    return ctx, alpha


# revision 9
# speedup vs baseline: 1.0475x; 1.0475x over previous
"""Additive (Bahdanau) attention on 8 TRN2 NeuronCores.

Reference:
    e_proj = einsum('tne,ae->tna', enc_out, W_e)          # [T,N,A]
    d_proj = einsum('nd,ad->na', dec_h, W_d)[None]        # [1,N,A]
    scores = einsum('tna,a->tn', tanh(e_proj + d_proj), v[0])
    alpha  = softmax(scores, axis=0)                      # over T
    ctx    = einsum('tn,tne->ne', alpha, enc_out)
    return ctx, alpha

Sharding: batch N=64 split 8 ways (n=8 per core), weights replicated, softmax
over T is core-local.  Device work per core (bf16 compute, f32 accumulate):
  phase A: e_projT chunks = W_eT.T @ encT  (PSUM), tanh(+d_proj bias) on ACT,
           scores via matmul against v packed in one-hot M-columns so group g
           lands on PSUM partition g -> scores live as [4, 512] per n
  softmax: exp straight off PSUM (scores bounded ~+-4, no max shift needed),
           free-dim partial sums, cross-partition total via a tiny ones-matmul,
           normalize on DVE
  phase B: ctx = alpha-weighted sum over T as K=128 matmuls on enc (T-major);
           the [4,512] alpha reshapes to the [128,16] lhsT via a DRAM bounce
           (Tile tracks the RAW dep through DRAM)
Host side only reshapes/transposes/casts inputs and glues shard outputs.
"""

import sys

for _p in ("/opt/trn_rl_repo", "/opt/pypackages"):
    if _p not in sys.path:
        sys.path.append(_p)

import numpy as np
import ml_dtypes
from contextlib import ExitStack

T, N, E, D, A = 2048, 64, 1024, 1024, 512
NC = 8          # cores
NL = N // NC    # batch per core = 8
P = 128         # partitions
EC = E // P     # 8 E-chunks
AC = A // P     # 4 A-chunks
DC = D // P     # 8 D-chunks
TG = 512        # t-group size for phase A
NG = T // TG    # 4 groups
TCH = T // P    # 16 t-chunks for phase B (K=128 each)

BF16 = ml_dtypes.bfloat16

_compiled = None


def _build():
    import concourse.bass as bass
    import concourse.tile as tile
    from concourse import bacc, mybir

    bf = mybir.dt.bfloat16
    f32 = mybir.dt.float32
    Act = mybir.ActivationFunctionType
    ts = bass.ts

    nc = bacc.Bacc()

    encT = nc.declare_dram_parameter("encT", [E, NL, T], bf, isOutput=False)
    encN = nc.declare_dram_parameter("encN", [NL, T, E], bf, isOutput=False)
    wet = nc.declare_dram_parameter("wet", [P, EC * A], bf, isOutput=False)
    wdt = nc.declare_dram_parameter("wdt", [P, DC * A], bf, isOutput=False)
    dech = nc.declare_dram_parameter("dech", [P, DC * NL], bf, isOutput=False)
    # v packed per (g, ac): [128, 4] slice = v[ac-chunk] in column g, else 0,
    # so the scores matmul for t-group g writes PSUM partition g.
    vg = nc.declare_dram_parameter("vg", [P, NG * AC * NG], bf, isOutput=False)
    ctx_out = nc.declare_dram_parameter("ctx", [NL, E], f32, isOutput=True)
    alphaT_out = nc.declare_dram_parameter("alphaT", [NL, T], f32, isOutput=True)

    abd = nc.dram_tensor("abd", [NL, T], bf)  # alpha bounce for reshape

    with tile.TileContext(nc) as tc, ExitStack() as ctx:
        wpool = ctx.enter_context(tc.tile_pool(name="w", bufs=1))
        etp = ctx.enter_context(tc.tile_pool(name="etp", bufs=5))
        enp = ctx.enter_context(tc.tile_pool(name="enp", bufs=5))
        thp = ctx.enter_context(tc.tile_pool(name="thp", bufs=6))
        scp = ctx.enter_context(tc.tile_pool(name="scp", bufs=2))
        alp = ctx.enter_context(tc.tile_pool(name="alp", bufs=2))
        ppe = ctx.enter_context(
            tc.tile_pool(name="ppe", bufs=4, space=bass.MemorySpace.PSUM)
        )
        pps = ctx.enter_context(
            tc.tile_pool(name="pps", bufs=2, space=bass.MemorySpace.PSUM)
        )
        ppc = ctx.enter_context(
            tc.tile_pool(name="ppc", bufs=2, space=bass.MemorySpace.PSUM)
        )

        # weights split across the two HWDGE rings so the startup chain
        # parallelizes: ACT ring feeds the first e_proj matmuls, SP ring
        # feeds d_proj.
        wdt_sb = wpool.tile([P, DC * A], bf)
        nc.scalar.dma_start(wdt_sb[:], wdt[:])
        dech_sb = wpool.tile([P, DC * NL], bf)
        nc.scalar.dma_start(dech_sb[:], dech[:])
        wet_sb = wpool.tile([P, EC * A], bf)
        nc.scalar.dma_start(wet_sb[:], wet[:])
        vg_sb = wpool.tile([P, NG * AC * NG], bf)
        nc.scalar.dma_start(vg_sb[:], vg[:])

        ones4 = wpool.tile([4, 4], f32)
        nc.vector.memset(ones4[:], 1.0)

        # --- d_projT[a, n] = (dec_h @ W_d.T).T, chunk ac at cols ac*NL ---
        dproj_sb = wpool.tile([P, AC * NL], f32)
        for ac in range(AC):
            dps = ppc.tile([P, NL], f32, tag="ppc")
            for dc in range(DC):
                nc.tensor.matmul(
                    dps[:],
                    wdt_sb[:, dc * A + ac * P : dc * A + (ac + 1) * P],
                    dech_sb[:, dc * NL : (dc + 1) * NL],
                    start=(dc == 0),
                    stop=(dc == DC - 1),
                )
            nc.scalar.activation(
                dproj_sb[:, ac * NL : (ac + 1) * NL], dps[:], Act.Copy
            )

        def phase_a(n):
            """scores for all T of batch-row n -> [4, TG] PSUM tile."""
            sps4 = pps.tile([NG, TG], f32, tag="pps")
            for g in range(NG):
                et = etp.tile([P, EC * TG], bf, tag="etp")
                src = encT[:, n, ts(g, TG)].rearrange("(a p) t -> p a t", p=P)
                nc.sync.dma_start(
                    et[:].rearrange("p (a t) -> p a t", t=TG), src
                )
                for ac in range(AC):
                    eps = ppe.tile([P, TG], f32, tag="ppe")
                    for ec in range(EC):
                        nc.tensor.matmul(
                            eps[:],
                            wet_sb[:, ec * A + ac * P : ec * A + (ac + 1) * P],
                            et[:, ts(ec, TG)],
                            start=(ec == 0),
                            stop=(ec == EC - 1),
                        )
                    th = thp.tile([P, TG], bf, tag="thp")
                    nc.scalar.activation(
                        th[:],
                        eps[:],
                        Act.Tanh,
                        bias=dproj_sb[:, ac * NL + n : ac * NL + n + 1],
                    )
                    nc.tensor.matmul(
                        sps4[:],
                        vg_sb[:, ts(g * AC + ac, NG)],
                        th[:],
                        start=(g == 0 and ac == 0),
                        stop=(g == NG - 1 and ac == AC - 1),
                    )
            return sps4

        def softmax(n, sps4):
            """alpha out to DRAM + [P, TCH] bf16 lhsT tile for phase B."""
            ex4 = scp.tile([NG, TG], f32, tag="ex")
            nc.scalar.activation(ex4[:], sps4[:], Act.Exp)
            l4 = alp.tile([NG, 1], f32, tag="l4")
            nc.vector.reduce_sum(l4[:], ex4[:], axis=mybir.AxisListType.X)
            ltp = ppc.tile([NG, 1], f32, tag="ppc")
            nc.tensor.matmul(ltp[:], ones4[:], l4[:], start=True, stop=True)
            lts = alp.tile([NG, 1], f32, tag="lts")
            nc.scalar.activation(lts[:], ltp[:], Act.Copy)
            r4 = alp.tile([NG, 1], f32, tag="r4")
            nc.vector.reciprocal(r4[:], lts[:])
            ab4 = scp.tile([NG, TG], bf, tag="ab")
            nc.vector.tensor_scalar_mul(ab4[:], ex4[:], r4[:])
            # reshape [4, TG] -> [P, TCH] via DRAM bounce; Tile tracks the
            # RAW dep through DRAM (read waits on the write's DMA sem).
            lt = alp.tile([P, TCH], bf, tag="lt")
            nc.gpsimd.dma_start(
                abd[n].rearrange("(g j) -> g j", g=NG), ab4[:]
            )
            nc.gpsimd.dma_start(lt[:], abd[n].rearrange("(p c) -> p c", p=P))
            al4 = scp.tile([NG, TG], f32, tag="al")
            nc.vector.tensor_scalar_mul(al4[:], ex4[:], r4[:])
            nc.gpsimd.dma_start(
                alphaT_out[n].rearrange("(g j) -> g j", g=NG), al4[:]
            )
            return lt

        def phase_b(n, lt):
            ens = []
            for g in range(NG):
                en = enp.tile([P, 4 * E], bf, tag="enp")
                src = encN[n].rearrange("(p c) e -> p c e", c=TCH)[
                    :, 4 * g : 4 * (g + 1), :
                ]
                nc.sync.dma_start(
                    en[:].rearrange("p (c e) -> p c e", e=E), src
                )
                ens.append(en)
            ctx_sb = alp.tile([1, E], f32, tag="cs")
            for h in range(2):
                cp = ppc.tile([1, 512], f32, tag="ppc")
                for c in range(TCH):
                    nc.tensor.matmul(
                        cp[:],
                        lt[:, c : c + 1],
                        ens[c // 4][
                            :, (c % 4) * E + h * 512 : (c % 4) * E + h * 512 + 512
                        ],
                        start=(c == 0),
                        stop=(c == TCH - 1),
                    )
                nc.scalar.activation(ctx_sb[0:1, ts(h, 512)], cp[:], Act.Copy)
            nc.gpsimd.dma_start(ctx_out[n : n + 1, :], ctx_sb[:])

        # software-pipelined by one n: PE order A(0) A(1) B(0) A(2) B(1) ...
        sps4 = phase_a(0)
        for n in range(NL):
            lt = softmax(n, sps4)
            if n + 1 < NL:
                sps4 = phase_a(n + 1)
            phase_b(n, lt)

    nc.compile()
    return nc


def _prep_core(enc_sl, dec_sl, W_e, W_d, v):
    """Build the per-core in_map (layout transforms + bf16 cast only)."""
    encT = np.ascontiguousarray(enc_sl.transpose(2, 1, 0)).astype(BF16)
    encN = np.ascontiguousarray(enc_sl.transpose(1, 0, 2)).astype(BF16)
    wet = np.ascontiguousarray(
        W_e.T.reshape(EC, P, A).transpose(1, 0, 2).reshape(P, EC * A)
    ).astype(BF16)
    wdt = np.ascontiguousarray(
        W_d.T.reshape(DC, P, A).transpose(1, 0, 2).reshape(P, DC * A)
    ).astype(BF16)
    dech = np.ascontiguousarray(
        dec_sl.T.reshape(DC, P, NL).transpose(1, 0, 2).reshape(P, DC * NL)
    ).astype(BF16)
    vcols = v[0].reshape(AC, P).T  # [128, ac]
    vgm = np.zeros((P, NG * AC * NG), dtype=np.float32)
    for g in range(NG):
        for ac in range(AC):
            vgm[:, (g * AC + ac) * NG + g] = vcols[:, ac]
    return {
        "encT": encT,
        "encN": encN,
        "wet": wet,
        "wdt": wdt,
        "dech": dech,
        "vg": vgm.astype(BF16),
    }


def kernel(enc_out, dec_h, W_e, W_d, v, _trace=False):
    global _compiled
    from concourse.bass_utils import run_bass_kernel_spmd

    if _compiled is None:
        _compiled = _build()
    nc = _compiled

    enc_out = np.asarray(enc_out, dtype=np.float32)
    dec_h = np.asarray(dec_h, dtype=np.float32)
    W_e = np.asarray(W_e, dtype=np.float32)
    W_d = np.asarray(W_d, dtype=np.float32)
    v = np.asarray(v, dtype=np.float32)

    in_maps = [
        _prep_core(
            enc_out[:, c * NL : (c + 1) * NL, :],
            dec_h[c * NL : (c + 1) * NL],
            W_e,
            W_d,
            v,
        )
        for c in range(NC)
    ]

    res = run_bass_kernel_spmd(nc, in_maps, list(range(NC)), trace=_trace)
    ctx = np.concatenate([np.asarray(r["ctx"]) for r in res.results], axis=0)
    alpha = np.concatenate(
        [np.asarray(r["alphaT"]).T for r in res.results], axis=1
    )
    if _trace:
        return (ctx, alpha), res
    return ctx, alpha
